# revision 1
# baseline (speedup 1.0000x reference)
"""Multi-head attention (B=2, S=2048, D=1024, H=16, dk=64) on 8 TRN2 cores.

Sharding: core c -> (batch b = c//4, head-group g = c%4 of 4 heads).
Each core computes q/k/v projections for its 4 heads, full attention for
those heads, and a partial output projection (rows g*256:(g+1)*256 of Wo).
Host pre-transposes/casts inputs to bf16 and sums the partial outputs.

Device layout (per core, all matmul operands bf16, accumulation f32):
  xqT/xkT/xvT [1024, 2048]   (d on partitions -> contraction-ready)
  qT, kT      [256, 2048]    (head-dim on partitions; pair tiles [128, S])
  v_aug       [2048, 4*65]   (per head: [v_h | ones]; ones col => softmax denom)
  scoresT     [j, i] in PSUM; exp on ScalarE -> probsT bf16 (no max-subtract:
              scores ~ N(0,1) after 1/8 scaling, exp bounded ~e^6)
  PV:         attnT_unnorm[e, i] = sum_j v_aug[j, e] * probsT[j, i]
              (row 64 = softmax denominator), normalize via reciprocal +
              K=1 broadcast matmul, store attnT [64, S] per head
  out-projT:  outT[n, s] = sum_{h,e} wo[h,e,n] * attnT_h[e, s]  (K=64 x4)
Host: out[b] = sum_g outT_partial.T + (bv @ Wo + bo).
"""

import os

import numpy as np
import ml_dtypes

BF16 = ml_dtypes.bfloat16

B, S, D = 2, 2048, 1024
H, DK = 16, 64
P = 128
GROUPS = 4          # head groups (one per core within a batch)
HPG = 4             # heads per group
GD = HPG * DK       # 256, group width
KC = D // P         # 8 contraction chunks
ST = S // P         # 16 s-tiles / j-tiles
NCORES = 8
FP8_PV = False      # fp8 PV measured 3.7e-2 rel err (e4m3 noise) - keep bf16
DEBUG_DUMP = False  # extra outputs: per-head attnT and denominators

_cached = {}


def _build_bass():
    import concourse.bass as bass
    import concourse.tile as tile
    from concourse.bacc import Bacc
    from concourse import mybir
    from contextlib import ExitStack

    f32 = mybir.dt.float32
    bf16 = mybir.dt.bfloat16
    Act = mybir.ActivationFunctionType

    nc = Bacc()

    xqT = nc.dram_tensor("xqT", [D, S], bf16, kind="ExternalInput")
    xkT = nc.dram_tensor("xkT", [D, S], bf16, kind="ExternalInput")
    xvT = nc.dram_tensor("xvT", [D, S], bf16, kind="ExternalInput")
    wq = nc.dram_tensor("wq", [D, GD], bf16, kind="ExternalInput")
    wk = nc.dram_tensor("wk", [D, GD], bf16, kind="ExternalInput")
    wv = nc.dram_tensor("wv", [D, GD], bf16, kind="ExternalInput")
    wo = nc.dram_tensor("wo", [GD, D], bf16, kind="ExternalInput")
    bq = nc.dram_tensor("bq", [GD, 1], f32, kind="ExternalInput")
    bk = nc.dram_tensor("bk", [GD, 1], f32, kind="ExternalInput")
    out = nc.dram_tensor("out", [S, D], f32, kind="ExternalOutput")

    with tile.TileContext(nc) as tc, ExitStack() as ctx:
        singles = ctx.enter_context(tc.tile_pool(name="singles", bufs=1))
        probs_pool = ctx.enter_context(tc.tile_pool(name="probs", bufs=3))
        small = ctx.enter_context(tc.tile_pool(name="small", bufs=8))
        outs_pool = ctx.enter_context(tc.tile_pool(name="outs", bufs=8))
        psum = ctx.enter_context(tc.tile_pool(name="psum", bufs=1, space="PSUM"))

        # ---- persistent SBUF ----
        wq_sb = singles.tile([P, KC, GD], bf16)
        wk_sb = singles.tile([P, KC, GD], bf16)
        wv_sb = singles.tile([P, KC, GD], bf16)
        wo_sb = singles.tile([P, 2, D], bf16)
        bq_sb = singles.tile([P, 2, 1], f32)
        bk_sb = singles.tile([P, 2, 1], f32)
        nc.sync.dma_start(out=wq_sb, in_=wq.rearrange("(c p) m -> p c m", p=P))
        nc.sync.dma_start(out=wk_sb, in_=wk.rearrange("(c p) m -> p c m", p=P))
        nc.sync.dma_start(out=wv_sb, in_=wv.rearrange("(c p) m -> p c m", p=P))
        nc.sync.dma_start(out=wo_sb, in_=wo.rearrange("(c p) n -> p c n", p=P))
        nc.sync.dma_start(out=bq_sb, in_=bq.rearrange("(t p) o -> p t o", p=P))
        nc.sync.dma_start(out=bk_sb, in_=bk.rearrange("(t p) o -> p t o", p=P))

        xq_sb = singles.tile([P, KC, S], bf16)
        xk_sb = singles.tile([P, KC, S], bf16)
        xv_sb = singles.tile([P, KC, S], bf16)
        # tensor-by-tensor so q-proj can start after the first xq chunk
        # and PE chases the DMA stream instead of waiting on all three
        for k in range(KC):
            nc.sync.dma_start(out=xq_sb[:, k, :], in_=xqT[k * P:(k + 1) * P, :])
        for k in range(KC):
            nc.sync.dma_start(out=xk_sb[:, k, :], in_=xkT[k * P:(k + 1) * P, :])
        for k in range(KC):
            nc.sync.dma_start(out=xv_sb[:, k, :], in_=xvT[k * P:(k + 1) * P, :])

        qT_sb = [singles.tile([P, S], bf16, name=f"qT{t}") for t in range(2)]
        kT_sb = [singles.tile([P, S], bf16, name=f"kT{t}") for t in range(2)]
        # attnT per head pair [128 hd, S]: even head at partitions 0:64
        # (written directly by DVE), odd head at 64:128 (DVE writes a base-0
        # staging tile, then SBUF->SBUF DMA relocates partitions - engines
        # are lane-locked but DMA is not). Enables K=128 out-projection.
        att_pair = [singles.tile([P, S], bf16, name=f"attp{p}")
                    for p in range(2)]
        att_odd = [singles.tile([DK, S], bf16, name=f"atto{p}")
                   for p in range(2)]

        ones_sb = singles.tile([65, DK], f32)
        nc.vector.memset(ones_sb[64:65, :], 1.0)

        CP = ST // 2
        if FP8_PV:
            fp8 = mybir.dt.float8e4
            # [j-in-chunk, chunk-pair, chunk-in-pair, head, 64 v cols + 1 one + pad]
            v_sb = singles.tile([P, CP, 2, HPG, 80], fp8)
            nc.vector.memset(v_sb[:, :, :, :, 64:65], 1.0)
            v4 = None
            # exp(s/8 - 3): keeps exp within IEEE e4m3 range (max finite 240;
            # max observed score ~7.7 -> e^4.7 ~ 110). Softmax shift-invariant.
            exp_bias = singles.tile([P, 1], f32)
            nc.vector.memset(exp_bias, -3.0)
        else:
            v_sb = singles.tile([P, ST, HPG * 65], bf16)
            # ones columns of v_aug (col 64 of each per-head [64|1] block)
            v4 = v_sb.rearrange("p s (h c) -> p s h c", c=65)
            nc.vector.memset(v4[:, :, :, 64:65], 1.0)

        # ---- phase A: projections ----
        def qk_proj(x_sb, w_sb, b_sb, dst, t):
            pq = [psum.tile([P, 1024], mybir.dt.float32, tag="sc", bufs=2,
                            name=f"pq{t}{half}") for half in range(2)]
            for k in range(KC):
                for half in range(2):
                    for sq in range(2):
                        nc.tensor.matmul(
                            out=pq[half][:, sq * 512:(sq + 1) * 512],
                            lhsT=w_sb[:, k, t * P:(t + 1) * P],
                            rhs=x_sb[:, k, half * 1024 + sq * 512:
                                     half * 1024 + (sq + 1) * 512],
                            start=(k == 0), stop=(k == KC - 1))
            for half in range(2):
                nc.vector.tensor_scalar_add(
                    out=dst[:, half * 1024:(half + 1) * 1024],
                    in0=pq[half], scalar1=b_sb[:, t, :])

        def v_proj():
            for st in range(ST):
                pvv = psum.tile([P, GD], mybir.dt.float32, tag="pv", bufs=4, name="pvv")
                for k in range(KC):
                    nc.tensor.matmul(
                        out=pvv,
                        lhsT=xv_sb[:, k, st * P:(st + 1) * P],
                        rhs=wv_sb[:, k, :],
                        start=(k == 0), stop=(k == KC - 1))
                if FP8_PV:
                    dst = v_sb[:, st // 2, st % 2, :, 0:64]
                else:
                    dst = v4[:, st, :, 0:64]
                src = pvv.rearrange("p (h c) -> p h c", c=64)
                nc.vector.tensor_copy(out=dst, in_=src)

        # ---- phase B: attention for one head pair, one i-half ----
        # `pending` = previous iteration's normalize emitter; it is emitted
        # after this iteration's first two j-tiles so ACT/PE stay fed across
        # the (pair, ih) boundary. Returns this iteration's normalize.
        def attention(pair, ih, pending=None):
            pv = [[psum.tile([65, 512], mybir.dt.float32, tag="pv", bufs=4,
                             name=f"pv{pair}{ih}{hp}{iq}")
                   for iq in range(2)] for hp in range(2)]
            if FP8_PV:
                fp8 = mybir.dt.float8e4
                for cp in range(CP):
                    pr = [probs_pool.tile([P, 2, 1024], fp8, tag="probs",
                                          name=f"pr{hp}") for hp in range(2)]
                    for d in range(2):
                        jt = 2 * cp + d
                        sc = [psum.tile([P, 1024], mybir.dt.float32, tag="sc",
                                        bufs=2, name=f"sc{hp}")
                              for hp in range(2)]
                        for iq in range(2):
                            for hp in range(2):
                                nc.tensor.matmul(
                                    out=sc[hp][:, iq * 512:(iq + 1) * 512],
                                    lhsT=kT_sb[pair][hp * 64:(hp + 1) * 64,
                                                     jt * P:(jt + 1) * P],
                                    rhs=qT_sb[pair][hp * 64:(hp + 1) * 64,
                                                    ih * 1024 + iq * 512:
                                                    ih * 1024 + (iq + 1) * 512],
                                    start=True, stop=True)
                        for hp in range(2):
                            # exp(s/8 - 2): global shift keeps exp within
                            # e4m3 range (softmax is shift-invariant)
                            nc.scalar.activation(out=pr[hp][:, d, :],
                                                 in_=sc[hp], func=Act.Exp,
                                                 scale=0.125, bias=exp_bias)
                    for hp in range(2):
                        h = 2 * pair + hp
                        for iq in range(2):
                            nc.tensor.matmul(
                                out=pv[hp][iq][:, :],
                                lhsT=v_sb[:, cp, :, h, 0:65],
                                rhs=pr[hp][:, :, iq * 512:(iq + 1) * 512],
                                perf_mode=mybir.MatmulPerfMode.DoubleRow,
                                start=(cp == 0), stop=(cp == CP - 1))
                    if cp == 1 and pending is not None:
                        pending()
            else:
                for jt in range(ST):
                    sc = [psum.tile([P, 1024], mybir.dt.float32, tag="sc",
                                    bufs=2, name=f"sc{hp}") for hp in range(2)]
                    for iq in range(2):
                        for hp in range(2):
                            nc.tensor.matmul(
                                out=sc[hp][:, iq * 512:(iq + 1) * 512],
                                lhsT=kT_sb[pair][hp * 64:(hp + 1) * 64,
                                                 jt * P:(jt + 1) * P],
                                rhs=qT_sb[pair][hp * 64:(hp + 1) * 64,
                                                ih * 1024 + iq * 512:
                                                ih * 1024 + (iq + 1) * 512],
                                start=True, stop=True)
                    for hp in range(2):
                        probs = probs_pool.tile([P, 1024], bf16, tag="probs",
                                                name="probs")
                        nc.scalar.activation(out=probs, in_=sc[hp],
                                             func=Act.Exp, scale=0.125)
                        h65 = (2 * pair + hp) * 65
                        for iq in range(2):
                            nc.tensor.matmul(
                                out=pv[hp][iq][:, :],
                                lhsT=v_sb[:, jt, h65:h65 + 65],
                                rhs=probs[:, iq * 512:(iq + 1) * 512],
                                start=(jt == 0), stop=(jt == ST - 1))
                    if jt == 1 and pending is not None:
                        pending()

            def normalize():
                for hp in range(2):
                    for iq in range(2):
                        r = small.tile([65, 512], mybir.dt.float32, tag="r",
                                       name="r")
                        nc.vector.reciprocal(out=r[64:65, :],
                                             in_=pv[hp][iq][64:65, :])
                        bc = psum.tile([64, 512], mybir.dt.float32, tag="pv",
                                       bufs=4, name="bc")
                        nc.tensor.matmul(out=bc, lhsT=ones_sb[64:65, :],
                                         rhs=r[64:65, :], start=True,
                                         stop=True)
                        pvs = small.tile([64, 512], mybir.dt.float32,
                                         tag="pvs", name="pvs")
                        nc.vector.tensor_copy(out=pvs, in_=pv[hp][iq][0:64, :])
                        col = ih * 1024 + iq * 512
                        if hp == 0:
                            nc.vector.tensor_mul(
                                out=att_pair[pair][0:64, col:col + 512],
                                in0=pvs, in1=bc)
                        else:
                            nc.vector.tensor_mul(
                                out=att_odd[pair][:, col:col + 512],
                                in0=pvs, in1=bc)
                            nc.sync.dma_start(
                                out=att_pair[pair][64:128, col:col + 512],
                                in_=att_odd[pair][:, col:col + 512])

            return normalize

        def out_proj():
            # out[s, n] = sum_c att_pair[c].T @ wo_chunk[c]  (K=128 per chunk)
            for st in range(ST):
                po = [psum.tile([P, 512], mybir.dt.float32, tag="pv", bufs=4,
                                name=f"po{nb}") for nb in range(2)]
                for c in range(2):
                    for nb in range(2):
                        nc.tensor.matmul(
                            out=po[nb],
                            lhsT=att_pair[c][:, st * P:(st + 1) * P],
                            rhs=wo_sb[:, c, nb * 512:(nb + 1) * 512],
                            start=(c == 0), stop=(c == 1))
                for nb in range(2):
                    osb = outs_pool.tile([P, 512], mybir.dt.float32,
                                         tag="osb", name="osb")
                    if nb % 2 == 0:
                        nc.vector.tensor_copy(out=osb, in_=po[nb])
                    else:
                        nc.scalar.copy(out=osb, in_=po[nb])
                    nc.sync.dma_start(
                        out=out[st * P:(st + 1) * P,
                                nb * 512:(nb + 1) * 512],
                        in_=osb)

        qk_proj(xq_sb, wq_sb, bq_sb, qT_sb[0], 0)
        qk_proj(xk_sb, wk_sb, bk_sb, kT_sb[0], 0)
        v_proj()
        qk_proj(xq_sb, wq_sb, bq_sb, qT_sb[1], 1)
        qk_proj(xk_sb, wk_sb, bk_sb, kT_sb[1], 1)
        pending = None
        for pair in range(2):
            for ih in range(2):
                pending = attention(pair, ih, pending)
        pending()
        out_proj()

    nc.finalize()
    return nc


def kernel(Q, K, V, Wq, bq, Wk, bk, Wv, bv, Wo, bo):
    from concourse.bass_utils import run_bass_kernel_spmd

    f32 = np.float32
    Q = np.asarray(Q, f32)
    K = np.asarray(K, f32)
    V = np.asarray(V, f32)
    Wq = np.asarray(Wq, f32)
    Wk = np.asarray(Wk, f32)
    Wv = np.asarray(Wv, f32)
    Wo = np.asarray(Wo, f32)
    bq = np.asarray(bq, f32)
    bk = np.asarray(bk, f32)
    bv = np.asarray(bv, f32)
    bo = np.asarray(bo, f32)

    xT = {}
    for b in range(B):
        xT[('q', b)] = np.ascontiguousarray(Q[b].T).astype(BF16)
        xT[('k', b)] = np.ascontiguousarray(K[b].T).astype(BF16)
        xT[('v', b)] = np.ascontiguousarray(V[b].T).astype(BF16)

    in_maps = []
    for c in range(NCORES):
        b, g = c // GROUPS, c % GROUPS
        sl = slice(g * GD, (g + 1) * GD)
        in_maps.append({
            "xqT": xT[('q', b)],
            "xkT": xT[('k', b)],
            "xvT": xT[('v', b)],
            "wq": np.ascontiguousarray(Wq[:, sl]).astype(BF16),
            "wk": np.ascontiguousarray(Wk[:, sl]).astype(BF16),
            "wv": np.ascontiguousarray(Wv[:, sl]).astype(BF16),
            "wo": np.ascontiguousarray(Wo[sl, :]).astype(BF16),
            "bq": np.ascontiguousarray(bq[sl].reshape(GD, 1)),
            "bk": np.ascontiguousarray(bk[sl].reshape(GD, 1)),
        })

    if "nc" not in _cached:
        _cached["nc"] = _build_bass()
    nc = _cached["nc"]

    try:
        res = run_bass_kernel_spmd(nc, in_maps, core_ids=list(range(NCORES)))
    except ModuleNotFoundError:
        # BASS_TRACE set but the axon ntff hook isn't shipped in this
        # container - retry untraced
        os.environ["BASS_NEVER_TRACE"] = "1"
        res = run_bass_kernel_spmd(nc, in_maps, core_ids=list(range(NCORES)))
    if res.exec_time_ns is not None:
        print(f"HW exec time: {res.exec_time_ns} ns")

    bo_eff = (bv @ Wo + bo).astype(f32)
    out = np.zeros((B, S, D), f32)
    for c in range(NCORES):
        b = c // GROUPS
        out[b] += res.results[c]["out"]
    out += bo_eff
    return out



# revision 38
# speedup vs baseline: 1.2719x; 1.2719x over previous
"""Multi-head attention (B=2, S=2048, D=1024, H=16, dk=64) on 8 TRN2 cores.

Sharding: core c -> (batch b = c//4, head-group g = c%4 of 4 heads).
Each core computes q/k/v projections for its 4 heads, full attention for
those heads, and a partial output projection (rows g*256:(g+1)*256 of Wo).
Host pre-transposes/casts inputs to bf16 and sums the partial outputs.

All matmuls bf16 (f32 accumulate).  fp8 was tried and measured: with
random Q/K the softmax is diffuse, attention output ~ mean(V) ~
sigma_v/sqrt(N_eff), so quantization noise on q/k/probs/v hits the
output at FULL relative strength (measured 4-7e-2 per fp8 stage vs the
2e-2 gate).  bf16 (~0.4%) is the floor precision.

Per-core dataflow:
  x chunks stream into SBUF rings (q/k) so projections chase the DMAs;
  weights arrive pre-transposed from the host (no strided-descriptor
  DMAs).  q/k projections -> PSUM -> bias-cast to qT/kT [128 (2 heads x
  64 d), S] bf16.
  Each (pair, ih) attention phase is two-pass over 16 j-tiles:
    pass A: 4 score matmuls (K=64, N=512) into one [128, 2048] PSUM tile
      per j-tile (bufs=2 -> a full tile of lookahead), then ONE exp instr
      per j-tile, alternating between ACT (native Exp) and DVE
      (Schraudolph: s*A+B -> int16 -> bitcast bf16, ~2% rms) into bf16
      probs [128, 2, 2048] kept in SBUF.
    pass B: PV bf16 matmuls densely into one [65, 2048] PSUM tile (4
      accumulator quadrants as slices; v_aug ones-column gives the
      softmax denominator row), staged to SBUF bf16 immediately (ACT+DVE
      halves) so the PSUM slot frees for the next phase's pass A.
  normalize (pending-style, emitted into the next phase): reciprocal of
  the denominator row (DVE), partition broadcast on the idle Pool
  engine, row-scale muls (DVE); attT bf16; odd head staged + SBUF DMA
  to partitions 64:128.
  out-projT: bf16, K=128 per head-pair chunk, PSUM -> bf16 osb (ACT+DVE
  half-copies) -> DRAM bf16 partials; host sums and adds bv@Wo + bo.
"""

import os

import numpy as np
import ml_dtypes

BF16 = ml_dtypes.bfloat16

B, S, D = 2, 2048, 1024
H, DK = 16, 64
P = 128
GROUPS = 4          # head groups (one per core within a batch)
HPG = 4             # heads per group
GD = HPG * DK       # 256, group width
KC = D // P         # 8 contraction chunks
ST = S // P         # 16 s-tiles / j-tiles
CP = ST // 2        # 8 j-tile pairs
NCORES = 8

LOG2E = 1.4426950408889634
EXP_A = 0.125 * LOG2E * 128.0     # bf16: 7 mantissa bits -> 128/octave
EXP_B = 127.0 * 128.0 - 5.54      # exponent bias - schraudolph shift (rint)

# tunable: of the 32 exp instrs per quarter-phase, how many go to the
# scalar (ACT) engine (exact exp; rest: DVE schraudolph ~2% rms).
EXP_ACT_Q = int(os.environ.get("KEXP_ACT", "21"))

_cached = {}


def _build_bass():
    import concourse.bass as bass
    import concourse.tile as tile
    from concourse.bacc import Bacc
    from concourse import mybir
    from contextlib import ExitStack

    f32 = mybir.dt.float32
    bf16 = mybir.dt.bfloat16
    i16 = mybir.dt.int16
    Act = mybir.ActivationFunctionType
    Alu = mybir.AluOpType

    nc = Bacc()

    xqT = nc.dram_tensor("xqT", [D, S], bf16, kind="ExternalInput")
    xkT = nc.dram_tensor("xkT", [D, S], bf16, kind="ExternalInput")
    xvT = nc.dram_tensor("xvT", [D, S], bf16, kind="ExternalInput")
    wq = nc.dram_tensor("wq", [P, KC, GD], bf16, kind="ExternalInput")
    wk = nc.dram_tensor("wk", [P, KC, GD], bf16, kind="ExternalInput")
    wv = nc.dram_tensor("wv", [P, KC, GD], bf16, kind="ExternalInput")
    wo = nc.dram_tensor("wo", [P, 2, D], bf16, kind="ExternalInput")
    bq = nc.dram_tensor("bq", [GD, 1], f32, kind="ExternalInput")
    bk = nc.dram_tensor("bk", [GD, 1], f32, kind="ExternalInput")
    out = nc.dram_tensor("out", [S, D], bf16, kind="ExternalOutput")

    with tile.TileContext(nc) as tc, ExitStack() as ctx:
        singles = ctx.enter_context(tc.tile_pool(name="singles", bufs=1))
        probs_pool = ctx.enter_context(tc.tile_pool(name="probs", bufs=8))
        small = ctx.enter_context(tc.tile_pool(name="small", bufs=2))
        outs_pool = ctx.enter_context(tc.tile_pool(name="outs", bufs=8))
        psum = ctx.enter_context(tc.tile_pool(name="psum", bufs=1, space="PSUM"))

        # ---- persistent SBUF ----
        wq_sb = singles.tile([P, KC, GD], bf16)
        wk_sb = singles.tile([P, KC, GD], bf16)
        wv_sb = singles.tile([P, KC, GD], bf16)
        wo_sb = singles.tile([P, 2, D], bf16)
        bq_sb = singles.tile([P, 2, 1], f32)
        bk_sb = singles.tile([P, 2, 1], f32)
        xv_sb = singles.tile([P, KC, S], bf16)

        xq_sb = singles.tile([P, KC, S], bf16)
        xk_sb = singles.tile([P, KC, S], bf16)
        nc.sync.dma_start(out=wq_sb, in_=wq[:, :, :])
        nc.sync.dma_start(out=bq_sb, in_=bq.rearrange("(t p) o -> p t o", p=P))
        for k in range(KC):
            nc.sync.dma_start(out=xq_sb[:, k, :], in_=xqT[k * P:(k + 1) * P, :])
        nc.scalar.dma_start(out=wk_sb, in_=wk[:, :, :])
        nc.scalar.dma_start(out=bk_sb,
                            in_=bk.rearrange("(t p) o -> p t o", p=P))
        for k in range(KC):
            nc.sync.dma_start(out=xk_sb[:, k, :],
                              in_=xkT[k * P:(k + 1) * P, :])
        nc.sync.dma_start(out=wv_sb, in_=wv[:, :, :])
        for k in range(KC):
            nc.sync.dma_start(out=xv_sb[:, k, :], in_=xvT[k * P:(k + 1) * P, :])
        nc.scalar.dma_start(out=wo_sb, in_=wo[:, :, :])

        # qT/kT bf16 [128 (2 heads x 64 d), S] per pair
        qT = [singles.tile([P, S], bf16, name=f"qT{t}") for t in range(2)]
        kT = [singles.tile([P, S], bf16, name=f"kT{t}") for t in range(2)]

        # v_aug bf16: [j-in-tile, jt, head, 64+1(+pad)]
        v_sb = singles.tile([P, ST, HPG, 66], bf16)
        nc.vector.memset(v_sb[:, :, :, 64:65], 1.0)

        # attT bf16 per (pair, ih): rows 0:64 even head, 64:128 odd head
        att = [[singles.tile([P, 1024], bf16, name=f"att{pr}{ih}")
                for ih in range(2)] for pr in range(2)]
        att_odd = [singles.tile([DK, 1024], bf16, name=f"atto{x}")
                   for x in range(2)]

        def qk_proj(x_sb, w_sb, b_sb, dst, t):
            pq = [psum.tile([P, 1024], f32, tag="pv", bufs=2,
                            name=f"pq{t}{half}") for half in range(2)]
            for k in range(KC):
                for half in range(2):
                    for sq in range(2):
                        nc.tensor.matmul(
                            out=pq[half][:, sq * 512:(sq + 1) * 512],
                            lhsT=w_sb[:, k, t * P:(t + 1) * P],
                            rhs=x_sb[:, k, half * 1024 + sq * 512:
                                     half * 1024 + (sq + 1) * 512],
                            start=(k == 0), stop=(k == KC - 1))
            for half in range(2):
                nc.vector.tensor_scalar_add(
                    out=dst[:, half * 1024:(half + 1) * 1024],
                    in0=pq[half], scalar1=b_sb[:, t, :])

        def v_proj_st(st):
            def emit():
                pvv = psum.tile([P, GD], f32, tag="sc", bufs=4, name="pvv")
                for k in range(KC):
                    nc.tensor.matmul(
                        out=pvv,
                        lhsT=xv_sb[:, k, st * P:(st + 1) * P],
                        rhs=wv_sb[:, k, :],
                        start=(k == 0), stop=(k == KC - 1))
                dst = v_sb[:, st, :, 0:64]
                src = pvv.rearrange("p (h c) -> p h c", c=64)
                nc.scalar.copy(out=dst, in_=src)
            return emit

        def attention(pair, ihq, exp_act, pending, fillers=()):
            """Quarter-phase for head pair `pair`, queries
            [ihq*512, (ihq+1)*512).  sc bufs=4 -> two j-tiles of exp
            lookahead; pv_t double-buffered -> no boundary stall."""
            ih, qc = ihq // 2, (ihq % 2) * 512
            icol = ihq * 512
            pv_t = psum.tile([65, 1024], f32, tag="pv", bufs=2, name="pvt")
            nexp = 0
            lag = []     # PV one (jt, hp) unit behind its exp
            fill = list(fillers)

            def do_pv(jt, hp, pr):
                h = 2 * pair + hp
                nc.tensor.matmul(
                    out=pv_t[:, hp * 512:(hp + 1) * 512],
                    lhsT=v_sb[:, jt, h, 0:65],
                    rhs=pr,
                    start=(jt == 0), stop=(jt == ST - 1))

            for jt in range(ST):
                for hp in range(2):
                    sc = psum.tile([P, 512], f32, tag="sc", bufs=4,
                                   name="sc")
                    nc.tensor.matmul(
                        out=sc,
                        lhsT=kT[pair][hp * 64:(hp + 1) * 64,
                                      jt * P:(jt + 1) * P],
                        rhs=qT[pair][hp * 64:(hp + 1) * 64,
                                     icol:icol + 512],
                        start=True, stop=True)
                    if fill:
                        fill.pop(0)()
                    pr = probs_pool.tile([P, 512], bf16, tag="probs",
                                         name="pr")
                    on_act = (((nexp + 1) * exp_act) // 32
                              - (nexp * exp_act) // 32)
                    if on_act:
                        nc.scalar.activation(out=pr, in_=sc,
                                             func=Act.Exp, scale=0.125)
                    else:
                        nc.vector.tensor_scalar(
                            out=pr.bitcast(i16), in0=sc,
                            scalar1=EXP_A, scalar2=EXP_B,
                            op0=Alu.mult, op1=Alu.add)
                    nexp += 1
                    lag.append((jt, hp, pr))
                    if len(lag) > 1:
                        do_pv(*lag.pop(0))
                if jt == 1 and pending is not None:
                    pending()
            while lag:
                do_pv(*lag.pop(0))

            def normalize():
                # reads pv_t (PSUM) directly: with bufs=2 the slot isn't
                # needed again until two sets later (~27us), far beyond
                # this chain's ~3us latency.
                with nc.allow_low_precision(reason="softmax denom in bf16"):
                    r = small.tile([1, 1024], bf16, tag="r", bufs=2, name="r")
                    nc.vector.reciprocal(out=r, in_=pv_t[64:65, :])
                rb = small.tile([64, 1024], bf16, tag="rb", bufs=2, name="rb")
                nc.gpsimd.partition_broadcast(rb, r)
                nc.vector.tensor_mul(out=att[pair][ih][0:64, qc:qc + 512],
                                     in0=pv_t[0:64, 0:512],
                                     in1=rb[:, 0:512])
                nc.vector.tensor_mul(out=att_odd[pair][:, qc:qc + 512],
                                     in0=pv_t[0:64, 512:1024],
                                     in1=rb[:, 512:1024])
                nc.sync.dma_start(out=att[pair][ih][64:128, qc:qc + 512],
                                  in_=att_odd[pair][:, qc:qc + 512])

            return normalize

        def out_proj(ihs):
            for st in [ihs * 8 + i for i in range(8)]:
                ih, sti = st // 8, st % 8
                osb = outs_pool.tile([P, 1024], bf16, tag="osb", name="osb")
                for nb in range(2):
                    po = psum.tile([P, 512], f32, tag="sc", bufs=4,
                                   name="po")
                    for c in range(2):
                        nc.tensor.matmul(
                            out=po,
                            lhsT=att[c][ih][:, sti * P:(sti + 1) * P],
                            rhs=wo_sb[:, c, nb * 512:(nb + 1) * 512],
                            start=(c == 0), stop=(c == 1))
                    if nb == 0:
                        nc.scalar.copy(out=osb[:, 0:512], in_=po)
                    else:
                        nc.vector.tensor_copy(out=osb[:, 512:1024], in_=po)
                nc.sync.dma_start(out=out[st * P:(st + 1) * P, :], in_=osb)

        # ---- schedule ----
        qk_proj(xq_sb, wq_sb, bq_sb, qT[0], 0)
        qk_proj(xq_sb, wq_sb, bq_sb, qT[1], 1)
        qk_proj(xk_sb, wk_sb, bk_sb, kT[0], 0)
        qk_proj(xk_sb, wk_sb, bk_sb, kT[1], 1)
        v_proj_st(0)()
        v_proj_st(1)()
        vfill = [v_proj_st(st) for st in range(2, ST)]

        phase = 0
        pending = None
        for pair in range(2):
            for ihq in range(4):
                pending = attention(pair, ihq, EXP_ACT_Q, pending,
                                    fillers=vfill if phase == 0 else ())
                phase += 1
        pending()
        out_proj(0)
        out_proj(1)

    nc.finalize()
    return nc


def kernel(Q, K, V, Wq, bq, Wk, bk, Wv, bv, Wo, bo):
    from concourse.bass_utils import run_bass_kernel_spmd

    f32 = np.float32
    Q = np.asarray(Q, f32)
    K = np.asarray(K, f32)
    V = np.asarray(V, f32)
    Wq = np.asarray(Wq, f32)
    Wk = np.asarray(Wk, f32)
    Wv = np.asarray(Wv, f32)
    Wo = np.asarray(Wo, f32)
    bq = np.asarray(bq, f32)
    bk = np.asarray(bk, f32)
    bv = np.asarray(bv, f32)
    bo = np.asarray(bo, f32)

    def pack_w(w):     # [D, GD] -> [128, KC, GD]
        return np.ascontiguousarray(
            w.reshape(KC, P, w.shape[1]).transpose(1, 0, 2)).astype(BF16)

    xT = {}
    for b in range(B):
        xT[('q', b)] = np.ascontiguousarray(Q[b].T).astype(BF16)
        xT[('k', b)] = np.ascontiguousarray(K[b].T).astype(BF16)
        xT[('v', b)] = np.ascontiguousarray(V[b].T).astype(BF16)

    in_maps = []
    for c in range(NCORES):
        b, g = c // GROUPS, c % GROUPS
        sl = slice(g * GD, (g + 1) * GD)
        wo_t = Wo[sl, :].reshape(2, P, D).transpose(1, 0, 2)
        in_maps.append({
            "xqT": xT[('q', b)],
            "xkT": xT[('k', b)],
            "xvT": xT[('v', b)],
            "wq": pack_w(Wq[:, sl]),
            "wk": pack_w(Wk[:, sl]),
            "wv": pack_w(Wv[:, sl]),
            "wo": np.ascontiguousarray(wo_t).astype(BF16),
            "bq": np.ascontiguousarray(bq[sl].reshape(GD, 1)),
            "bk": np.ascontiguousarray(bk[sl].reshape(GD, 1)),
        })

    if "nc" not in _cached:
        _cached["nc"] = _build_bass()
    nc = _cached["nc"]

    try:
        res = run_bass_kernel_spmd(nc, in_maps, core_ids=list(range(NCORES)))
    except ModuleNotFoundError:
        os.environ["BASS_NEVER_TRACE"] = "1"
        res = run_bass_kernel_spmd(nc, in_maps, core_ids=list(range(NCORES)))
    if res.exec_time_ns is not None:
        print(f"HW exec time: {res.exec_time_ns} ns")

    bo_eff = (bv @ Wo + bo).astype(f32)
    out = np.zeros((B, S, D), f32)
    for c in range(NCORES):
        b = c // GROUPS
        out[b] += res.results[c]["out"].astype(f32)
    out += bo_eff
    return out
